# revision 12
# baseline (speedup 1.0000x reference)
"""Trainium2 Bass kernel for nn_AttentionBlock (GroupNorm + MHA + out-proj + residual).

Sharding: pure data-parallel over batch B=16 across 8 NeuronCores (2 per core).
Each core runs the identical program on its 2 batch elements; no collectives.

Per-core pipeline (L=1024 tokens, C=512 channels, 8 heads x 64):
  1. DMA x tiles [128 tok, 512 C], PE-transpose to x^T [C, L] layout.
  2. GroupNorm: bn_stats per channel over L, tiny PE matmuls aggregate/broadcast
     per-group stats (32 groups of 16 channels), affine apply on DVE.
  3. QKV: q,k produced transposed [feat, tok] (head h lives at partition base
     (h%2)*64 enabling 2-way PE row-packing of the K=64 score matmuls);
     v produced in [tok, feat] layout. q/k/v stored bf16.
  4. Attention per head without max-subtraction (scores ~N(0,1); exp safe):
     S^T tiles = k_tile^T.T @ q^T (f32 PSUM), exp on ScalarE (scale=1/8 fused),
     out^T accumulated in PSUM via [v]: rows 0-63 and a col-tiled ones matmul
     that writes 64 denominator replicas into rows 64-127 (concurrent, free).
  5. Normalize with DVE reciprocal+multiply, out-projection (f32r), +bias,
     +residual (GPSIMD), DMA out.
"""
import os
import sys

for _p in ("/opt/trn_rl_repo",):
    if _p not in sys.path and os.path.isdir(_p):
        sys.path.insert(0, _p)

import numpy as np

import concourse.bass as bass
import concourse.bacc as bacc
import concourse.mybir as mybir
import concourse.tile as tile
from concourse.masks import make_identity

F32 = mybir.dt.float32
F32R = mybir.dt.float32r
BF16 = mybir.dt.bfloat16

B_LOCAL = 2        # batch elements per core
L = 1024           # tokens (H*W)
C = 512            # channels
NH = 8             # heads
D = 64             # head dim
GROUPS = 32
GSIZE = C // GROUPS  # 16
EPS = 1e-5
NCHUNK = C // 128    # 4 channel chunks
NTT = L // 128       # 8 token tiles
SCALE = 1.0 / 8.0    # (1/sqrt(sqrt(64)))**2 applied inside exp


def r32(ap):
    return ap.bitcast(F32R)


def build_attention_block(tc, ctx):
    nc = tc.nc
    AF = mybir.ActivationFunctionType
    OP = mybir.AluOpType

    x_d = nc.dram_tensor("x", [B_LOCAL, L, C], F32, kind="ExternalInput").ap()
    gamma_d = nc.dram_tensor("gamma", [C], F32, kind="ExternalInput").ap()
    beta_d = nc.dram_tensor("beta", [C], F32, kind="ExternalInput").ap()
    wq_d = nc.dram_tensor("w_qkv", [C, 3 * C], F32R, kind="ExternalInput").ap()
    bq_d = nc.dram_tensor("b_qkv", [3 * C], F32, kind="ExternalInput").ap()
    wo_d = nc.dram_tensor("w_out", [C, C], F32R, kind="ExternalInput").ap()
    bo_d = nc.dram_tensor("b_out", [C], F32, kind="ExternalInput").ap()
    out_d = nc.dram_tensor("out", [B_LOCAL, L, C], F32, kind="ExternalOutput").ap()

    singles = ctx.enter_context(tc.tile_pool(name="singles", bufs=1))
    xin = ctx.enter_context(tc.tile_pool(name="xin", bufs=8))
    big = ctx.enter_context(tc.tile_pool(name="big", bufs=2))
    small = ctx.enter_context(tc.tile_pool(name="small", bufs=3))
    epool = ctx.enter_context(tc.tile_pool(name="epool", bufs=4))
    rpool = ctx.enter_context(tc.tile_pool(name="rpool", bufs=3))
    hpool = ctx.enter_context(tc.tile_pool(name="hpool", bufs=3))
    xres = ctx.enter_context(tc.tile_pool(name="xres", bufs=3))
    pscore = ctx.enter_context(tc.tile_pool(name="pscore", bufs=2, space="PSUM"))
    paout = ctx.enter_context(tc.tile_pool(name="paout", bufs=2, space="PSUM"))
    pmm = ctx.enter_context(tc.tile_pool(name="pmm", bufs=2, space="PSUM"))

    # ---- one-time constants ----
    identity = singles.tile([128, 128], F32)
    make_identity(nc, identity)

    # e_mat[c, g] = 1 iff c//16 == g (built as a band via two affine selects)
    e_mat = singles.tile([128, 8], F32)       # channel -> group indicator
    nc.gpsimd.memset(e_mat, 1.0)
    nc.gpsimd.affine_select(out=e_mat, in_=e_mat, compare_op=mybir.AluOpType.is_ge,
                            fill=0.0, base=0, pattern=[[-GSIZE, 8]],
                            channel_multiplier=1)
    nc.gpsimd.affine_select(out=e_mat, in_=e_mat, compare_op=mybir.AluOpType.is_ge,
                            fill=0.0, base=GSIZE - 1, pattern=[[GSIZE, 8]],
                            channel_multiplier=-1)
    e2_mat = singles.tile([8, 128], F32)      # group -> channel indicator
    nc.gpsimd.memset(e2_mat, 1.0)
    nc.gpsimd.affine_select(out=e2_mat, in_=e2_mat, compare_op=mybir.AluOpType.is_ge,
                            fill=0.0, base=0, pattern=[[1, 128]],
                            channel_multiplier=-GSIZE)
    nc.gpsimd.affine_select(out=e2_mat, in_=e2_mat, compare_op=mybir.AluOpType.is_ge,
                            fill=0.0, base=GSIZE - 1, pattern=[[-1, 128]],
                            channel_multiplier=GSIZE)

    ones64 = singles.tile([128, 64], BF16)
    nc.vector.memset(ones64, 1.0)

    eps_sb = singles.tile([8, 1], F32)
    nc.vector.memset(eps_sb, EPS)

    wq_sb = singles.tile([128, NCHUNK, 3 * C], F32R)
    nc.sync.dma_start(wq_sb, wq_d.rearrange("(o p) f -> p o f", p=128))
    wo_sb = singles.tile([128, NCHUNK, C], F32R)
    nc.sync.dma_start(wo_sb, wo_d.rearrange("(o p) f -> p o f", p=128))
    gamma_sb = singles.tile([128, NCHUNK], F32)
    nc.sync.dma_start(gamma_sb, gamma_d.rearrange("(o p) -> p o", p=128))
    beta_sb = singles.tile([128, NCHUNK], F32)
    nc.sync.dma_start(beta_sb, beta_d.rearrange("(o p) -> p o", p=128))
    bqk_sb = singles.tile([128, 8], F32)      # q,k biases per [partition, fi]
    nc.sync.dma_start(bqk_sb, bq_d[0:2 * C].rearrange("(o p) -> p o", p=128))
    bv_bc = singles.tile([128, C], F32)       # v bias broadcast across partitions
    nc.sync.dma_start(bv_bc, bq_d[2 * C:3 * C].partition_broadcast(128))
    bo_bc = singles.tile([128, C], F32)
    nc.sync.dma_start(bo_bc, bo_d.partition_broadcast(128))

    # ---- per-batch persistent tiles ----
    def phase_transpose_gn(b):
        """Produce group-normed x^T [128, chunk, L] (fp32) for batch b."""
        xT = big.tile([128, NCHUNK, L], F32R, tag="xT")
        x_tiles = []
        for tt in range(NTT):
            xt = xin.tile([128, C], F32, tag="x_in")
            nc.sync.dma_start(xt, x_d[b, tt * 128:(tt + 1) * 128, :])
            x_tiles.append(xt)
        for cc in range(NCHUNK):
            for half in range(2):
                tp = pmm.tile([128, 512], F32, tag="mm")
                for j in range(4):
                    tt = half * 4 + j
                    # out = x_chunk.T @ I — a plain matmul transpose (the
                    # transpose-mode S3_LW encoding has only 1 wait slot).
                    nc.tensor.matmul(
                        tp[:, j * 128:(j + 1) * 128],
                        lhsT=x_tiles[tt][:, cc * 128:(cc + 1) * 128],
                        rhs=identity,
                        start=True, stop=True,
                    )
                nc.vector.tensor_copy(xT[:, cc, half * 512:(half + 1) * 512], tp)

        for cc in range(NCHUNK):
            st = small.tile([128, 2, 6], F32, tag="bnst")
            for s in range(2):
                nc.vector.bn_stats(st[:, s], xT[:, cc, s * 512:(s + 1) * 512].bitcast(F32))
            mv = small.tile([128, 2], F32, tag="mv")
            nc.vector.bn_aggr(mv, st)
            # sq = [mean_c, var_c + mean_c^2]
            sq = small.tile([128, 2], F32, tag="sq")
            nc.vector.tensor_copy(sq[:, 0:1], mv[:, 0:1])
            nc.vector.tensor_tensor(sq[:, 1:2], mv[:, 0:1], mv[:, 0:1], op=OP.mult)
            nc.vector.tensor_tensor(sq[:, 1:2], sq[:, 1:2], mv[:, 1:2], op=OP.add)
            # group sums over the 8 groups in this chunk
            gs = pmm.tile([8, 2], F32, tag="mm")
            nc.tensor.matmul(gs, lhsT=e_mat, rhs=sq, start=True, stop=True)
            gsb = small.tile([8, 2], F32, tag="gsb")
            nc.vector.tensor_scalar_mul(gsb, gs, 1.0 / GSIZE)  # [m_g, E[x^2]_g]
            var = small.tile([8, 1], F32, tag="var")
            nc.vector.tensor_tensor(var, gsb[:, 0:1], gsb[:, 0:1], op=OP.mult)
            nc.vector.tensor_tensor(var, gsb[:, 1:2], var, op=OP.subtract)
            std = small.tile([8, 1], F32, tag="std")
            nc.scalar.activation(std, var, AF.Sqrt, bias=eps_sb)
            nc.vector.reciprocal(gsb[:, 1:2], std)             # [m_g, rstd_g]
            # broadcast group stats back to channels
            bc = pmm.tile([128, 2], F32, tag="mm")
            nc.tensor.matmul(bc, lhsT=e2_mat, rhs=gsb, start=True, stop=True)
            ab = small.tile([128, 2], F32, tag="ab")
            nc.vector.tensor_tensor(ab[:, 0:1], bc[:, 1:2],
                                    gamma_sb[:, cc:cc + 1], op=OP.mult)
            nc.vector.tensor_tensor(ab[:, 1:2], bc[:, 0:1], ab[:, 0:1], op=OP.mult)
            nc.vector.tensor_tensor(ab[:, 1:2], beta_sb[:, cc:cc + 1],
                                    ab[:, 1:2], op=OP.subtract)
            nc.vector.tensor_scalar(out=xT[:, cc, :], in0=xT[:, cc, :].bitcast(F32),
                                    scalar1=ab[:, 0:1], scalar2=ab[:, 1:2],
                                    op0=OP.mult, op1=OP.add)
        return xT

    def phase_qkv(xT):
        """qkT [128, 8, L] bf16 (features f=fi*128+p on partitions; q fi 0-3,
        k fi 4-7) and v [128, tt, C] bf16 ([tok, feat])."""
        qkT = big.tile([128, 8, L], BF16, tag="qkT")
        v_sb = big.tile([128, NTT, C], BF16, tag="v")
        for fi in range(8):
            for tb in range(2):
                ps = pmm.tile([128, 512], F32, tag="mm")
                for kc in range(NCHUNK):
                    nc.tensor.matmul(
                        ps,
                        lhsT=wq_sb[:, kc, fi * 128:(fi + 1) * 128],
                        rhs=xT[:, kc, tb * 512:(tb + 1) * 512],
                        start=(kc == 0), stop=(kc == NCHUNK - 1),
                    )
                nc.vector.tensor_scalar(
                    out=qkT[:, fi, tb * 512:(tb + 1) * 512], in0=ps,
                    scalar1=bqk_sb[:, fi:fi + 1], scalar2=None, op0=OP.add)
        for tt in range(NTT):
            ps = pmm.tile([128, 512], F32, tag="mm")
            for kc in range(NCHUNK):
                nc.tensor.matmul(
                    ps,
                    lhsT=xT[:, kc, tt * 128:(tt + 1) * 128],
                    rhs=wq_sb[:, kc, 2 * C:3 * C],
                    start=(kc == 0), stop=(kc == NCHUNK - 1),
                )
            nc.vector.tensor_tensor(out=v_sb[:, tt, :], in0=ps, in1=bv_bc, op=OP.add)
        return qkT, v_sb

    def phase_attention(qkT, v_sb):
        """attn_outT [128, chunk, L] fp32: feature f=kc*128+p, head=2*kc+(p>=64)."""
        aT = big.tile([128, NCHUNK, L], F32R, tag="attnT")
        for hp in range(4):
            h0, h1 = 2 * hp, 2 * hp + 1
            qT0 = qkT[0:64, hp, :]
            kT0 = qkT[0:64, 4 + hp, :]
            qT1 = qkT[64:128, hp, :]
            kT1 = qkT[64:128, 4 + hp, :]
            for qb in range(2):
                qs = slice(qb * 512, (qb + 1) * 512)
                out0 = paout.tile([128, 512], F32, tag="aout")
                out1 = paout.tile([128, 512], F32, tag="aout")
                for g in range(4):
                    s0 = pscore.tile([128, 2, 512], F32, tag="sc")
                    s1 = pscore.tile([128, 2, 512], F32, tag="sc")
                    for j in range(2):
                        kt = 2 * g + j
                        ks = slice(kt * 128, (kt + 1) * 128)
                        nc.tensor.matmul(s0[:, j], lhsT=kT0[:, ks], rhs=qT0[:, qs],
                                         start=True, stop=True)
                        nc.tensor.matmul(s1[:, j], lhsT=kT1[:, ks], rhs=qT1[:, qs],
                                         start=True, stop=True)
                    e0 = epool.tile([128, 2, 512], BF16, tag="e")
                    e1 = epool.tile([128, 2, 512], BF16, tag="e")
                    nc.scalar.activation(e0, s0, AF.Exp, scale=SCALE)
                    nc.scalar.activation(e1, s1, AF.Exp, scale=SCALE)
                    for j in range(2):
                        kt = 2 * g + j
                        for (ops, vh, eh) in ((out0, h0, e0), (out1, h1, e1)):
                            nc.tensor.matmul(
                                ops[0:64], lhsT=v_sb[:, kt, vh * 64:(vh + 1) * 64],
                                rhs=eh[:, j], start=(kt == 0), stop=(kt == 7),
                                skip_group_check=True)
                            nc.tensor.matmul(
                                ops[64:128], lhsT=ones64, rhs=eh[:, j],
                                start=(kt == 0), stop=(kt == 7),
                                tile_position=(0, 64), skip_group_check=True)
                rc0 = rpool.tile([64, 512], F32, tag="rc")
                nc.vector.reciprocal(rc0, out0[64:128])
                nc.vector.tensor_tensor(out=aT[0:64, hp, qs], in0=out0[0:64],
                                        in1=rc0, op=OP.mult)
                rc1 = rpool.tile([64, 512], F32, tag="rc")
                nc.vector.reciprocal(rc1, out1[64:128])
                nc.vector.tensor_tensor(out=aT[64:128, hp, qs], in0=out1[0:64],
                                        in1=rc1, op=OP.mult)
        return aT

    def phase_proj(b, aT):
        for tt in range(NTT):
            ps = pmm.tile([128, 512], F32, tag="mm")
            for kc in range(NCHUNK):
                nc.tensor.matmul(
                    ps,
                    lhsT=aT[:, kc, tt * 128:(tt + 1) * 128],
                    rhs=wo_sb[:, kc, :],
                    start=(kc == 0), stop=(kc == NCHUNK - 1),
                )
            xr = xres.tile([128, C], F32, tag="xres")
            nc.sync.dma_start(xr, x_d[b, tt * 128:(tt + 1) * 128, :])
            hh = hpool.tile([128, C], F32, tag="h")
            nc.vector.tensor_tensor(out=hh, in0=ps, in1=bo_bc, op=OP.add)
            nc.gpsimd.tensor_tensor(out=hh, in0=hh, in1=xr, op=OP.add)
            nc.sync.dma_start(out_d[b, tt * 128:(tt + 1) * 128, :], hh)

    xTs = [phase_transpose_gn(b) for b in range(B_LOCAL)]
    qkvs = [phase_qkv(xTs[b]) for b in range(B_LOCAL)]
    for b in range(B_LOCAL):
        aT = phase_attention(*qkvs[b])
        phase_proj(b, aT)


_NC_CACHE = None


def _get_nc():
    global _NC_CACHE
    if _NC_CACHE is None:
        from contextlib import ExitStack

        nc = bacc.Bacc("TRN2", target_bir_lowering=False, debug=False)
        with tile.TileContext(nc) as tc, ExitStack() as ctx:
            build_attention_block(tc, ctx)
        nc.compile()
        _NC_CACHE = nc
    return _NC_CACHE


def run(inputs, trace=False, tmpdir=None):
    """Run on 8 NeuronCores. Returns (full_output, BassKernelResults)."""
    from concourse import bass_utils

    x = np.ascontiguousarray(np.asarray(inputs["x"], dtype=np.float32))
    B, H, W, Cc = x.shape
    xs = x.reshape(B, H * W, Cc)
    common = {
        "gamma": np.ascontiguousarray(np.asarray(inputs["gamma"], np.float32)),
        "beta": np.ascontiguousarray(np.asarray(inputs["beta"], np.float32)),
        "w_qkv": np.ascontiguousarray(np.asarray(inputs["w_qkv"], np.float32)),
        "b_qkv": np.ascontiguousarray(np.asarray(inputs["b_qkv"], np.float32)),
        "w_out": np.ascontiguousarray(np.asarray(inputs["w_out"], np.float32)),
        "b_out": np.ascontiguousarray(np.asarray(inputs["b_out"], np.float32)),
    }
    n_cores = 8
    per = B // n_cores
    in_maps = [
        {"x": np.ascontiguousarray(xs[c * per:(c + 1) * per]), **common}
        for c in range(n_cores)
    ]
    nc = _get_nc()
    res = bass_utils.run_bass_kernel_spmd(
        nc, in_maps, core_ids=list(range(n_cores)), trace=trace, tmpdir=tmpdir)
    out = np.concatenate([r["out"] for r in res.results], axis=0)
    return out.reshape(B, H, W, Cc), res


def kernel(**inputs):
    out, _ = run(inputs, trace=False)
    return out


# revision 18
# speedup vs baseline: 1.4163x; 1.4163x over previous
"""Trainium2 Bass kernel for nn_AttentionBlock (GroupNorm + MHA + out-proj + residual).

Sharding: pure data-parallel over batch B=16 across 8 NeuronCores (2 per core).
Each core runs the identical program on its 2 batch elements; no collectives.

Per-core pipeline (L=1024 tokens, C=512 channels, 8 heads x 64):
  1. DMA x tiles [128 tok, 512 C], PE-transpose to x^T [C, L] layout.
  2. GroupNorm: bn_stats per channel over L, tiny PE matmuls aggregate/broadcast
     per-group stats (32 groups of 16 channels), affine apply on DVE.
  3. QKV: q,k produced transposed [feat, tok] (head h lives at partition base
     (h%2)*64 enabling 2-way PE row-packing of the K=64 score matmuls);
     v produced in [tok, feat] layout. q/k/v stored bf16.
  4. Attention per head without max-subtraction (scores ~N(0,1); exp safe):
     S^T tiles = k_tile^T.T @ q^T (f32 PSUM), exp on ScalarE (scale=1/8 fused),
     out^T accumulated in PSUM via [v]: rows 0-63 and a col-tiled ones matmul
     that writes 64 denominator replicas into rows 64-127 (concurrent, free).
  5. Normalize with DVE reciprocal+multiply, out-projection (f32r), +bias,
     +residual (GPSIMD), DMA out.
"""
import os
import sys

for _p in ("/opt/trn_rl_repo",):
    if _p not in sys.path and os.path.isdir(_p):
        sys.path.insert(0, _p)

import numpy as np

import concourse.bass as bass
import concourse.bacc as bacc
import concourse.mybir as mybir
import concourse.tile as tile
from concourse.masks import make_identity

F32 = mybir.dt.float32
F32R = mybir.dt.float32r
BF16 = mybir.dt.bfloat16

B_LOCAL = 2        # batch elements per core
L = 1024           # tokens (H*W)
C = 512            # channels
NH = 8             # heads
D = 64             # head dim
GROUPS = 32
GSIZE = C // GROUPS  # 16
EPS = 1e-5
NCHUNK = C // 128    # 4 channel chunks
NTT = L // 128       # 8 token tiles
SCALE = 1.0 / 8.0    # (1/sqrt(sqrt(64)))**2 applied inside exp


def r32(ap):
    return ap.bitcast(F32R)


def build_attention_block(tc, ctx):
    nc = tc.nc
    AF = mybir.ActivationFunctionType
    OP = mybir.AluOpType

    x_d = nc.dram_tensor("x", [B_LOCAL, L, C], F32, kind="ExternalInput").ap()
    gamma_d = nc.dram_tensor("gamma", [C], F32, kind="ExternalInput").ap()
    beta_d = nc.dram_tensor("beta", [C], F32, kind="ExternalInput").ap()
    wq_d = nc.dram_tensor("w_qkv", [C, 3 * C], F32R, kind="ExternalInput").ap()
    bq_d = nc.dram_tensor("b_qkv", [3 * C], F32, kind="ExternalInput").ap()
    wo_d = nc.dram_tensor("w_out", [C, C], F32R, kind="ExternalInput").ap()
    bo_d = nc.dram_tensor("b_out", [C], F32, kind="ExternalInput").ap()
    out_d = nc.dram_tensor("out", [B_LOCAL, L, C], F32, kind="ExternalOutput").ap()

    singles = ctx.enter_context(tc.tile_pool(name="singles", bufs=1))
    xin = ctx.enter_context(tc.tile_pool(name="xin", bufs=8))
    big = ctx.enter_context(tc.tile_pool(name="big", bufs=2))
    small = ctx.enter_context(tc.tile_pool(name="small", bufs=3))
    epool = ctx.enter_context(tc.tile_pool(name="epool", bufs=4))
    rpool = ctx.enter_context(tc.tile_pool(name="rpool", bufs=3))
    hpool = ctx.enter_context(tc.tile_pool(name="hpool", bufs=3))
    pscore = ctx.enter_context(tc.tile_pool(name="pscore", bufs=2, space="PSUM"))
    paout = ctx.enter_context(tc.tile_pool(name="paout", bufs=2, space="PSUM"))
    pmm = ctx.enter_context(tc.tile_pool(name="pmm", bufs=2, space="PSUM"))

    # ---- one-time constants ----
    identity = singles.tile([128, 128], F32)
    make_identity(nc, identity)

    # e_mat[c, g] = 1 iff c//16 == g (built as a band via two affine selects)
    e_mat = singles.tile([128, 8], F32)       # channel -> group indicator
    nc.gpsimd.memset(e_mat, 1.0)
    nc.gpsimd.affine_select(out=e_mat, in_=e_mat, compare_op=mybir.AluOpType.is_ge,
                            fill=0.0, base=0, pattern=[[-GSIZE, 8]],
                            channel_multiplier=1)
    nc.gpsimd.affine_select(out=e_mat, in_=e_mat, compare_op=mybir.AluOpType.is_ge,
                            fill=0.0, base=GSIZE - 1, pattern=[[GSIZE, 8]],
                            channel_multiplier=-1)
    e2_mat = singles.tile([8, 128], F32)      # group -> channel indicator
    nc.gpsimd.memset(e2_mat, 1.0)
    nc.gpsimd.affine_select(out=e2_mat, in_=e2_mat, compare_op=mybir.AluOpType.is_ge,
                            fill=0.0, base=0, pattern=[[1, 128]],
                            channel_multiplier=-GSIZE)
    nc.gpsimd.affine_select(out=e2_mat, in_=e2_mat, compare_op=mybir.AluOpType.is_ge,
                            fill=0.0, base=GSIZE - 1, pattern=[[-1, 128]],
                            channel_multiplier=GSIZE)

    eps_sb = singles.tile([8, 1], F32)
    nc.vector.memset(eps_sb, EPS)

    wq_sb = singles.tile([128, NCHUNK, 3 * C], F32R)
    nc.sync.dma_start(wq_sb, wq_d.rearrange("(o p) f -> p o f", p=128))
    wo_f32 = singles.tile([128, NCHUNK, C], F32)
    nc.sync.dma_start(wo_f32, wo_d.rearrange("(o p) f -> p o f", p=128).bitcast(F32))
    wo_sb = singles.tile([128, NCHUNK, C], BF16)
    nc.vector.tensor_copy(wo_sb, wo_f32)
    gamma_sb = singles.tile([128, NCHUNK], F32)
    nc.sync.dma_start(gamma_sb, gamma_d.rearrange("(o p) -> p o", p=128))
    beta_sb = singles.tile([128, NCHUNK], F32)
    nc.sync.dma_start(beta_sb, beta_d.rearrange("(o p) -> p o", p=128))
    bqk_sb = singles.tile([128, 8], F32)      # q,k biases per [partition, fi]
    nc.sync.dma_start(bqk_sb, bq_d[0:2 * C].rearrange("(o p) -> p o", p=128))
    bv_bc = singles.tile([128, C], F32)       # v bias broadcast across partitions
    nc.sync.dma_start(bv_bc, bq_d[2 * C:3 * C].partition_broadcast(128))
    bo_bc = singles.tile([128, C], F32)
    nc.sync.dma_start(bo_bc, bo_d.partition_broadcast(128))

    # ---- per-batch persistent tiles ----
    def phase_transpose_gn(b):
        """Produce group-normed x^T [128, chunk, L] (fp32) for batch b."""
        xT = big.tile([128, NCHUNK, L], F32R, tag="xT")
        x_tiles = []
        for tt in range(NTT):
            xt = xin.tile([128, C], F32, tag="x_in")
            nc.sync.dma_start(xt, x_d[b, tt * 128:(tt + 1) * 128, :])
            x_tiles.append(xt)
        for cc in range(NCHUNK):
            for half in range(2):
                tp = pmm.tile([128, 512], F32, tag="mm")
                for j in range(4):
                    tt = half * 4 + j
                    # out = x_chunk.T @ I — a plain matmul transpose (the
                    # transpose-mode S3_LW encoding has only 1 wait slot).
                    nc.tensor.matmul(
                        tp[:, j * 128:(j + 1) * 128],
                        lhsT=x_tiles[tt][:, cc * 128:(cc + 1) * 128],
                        rhs=identity,
                        start=True, stop=True,
                    )
                nc.vector.tensor_copy(xT[:, cc, half * 512:(half + 1) * 512], tp)

        for cc in range(NCHUNK):
            st = small.tile([128, 2, 6], F32, tag="bnst")
            for s in range(2):
                nc.vector.bn_stats(st[:, s], xT[:, cc, s * 512:(s + 1) * 512].bitcast(F32))
            mv = small.tile([128, 2], F32, tag="mv")
            nc.vector.bn_aggr(mv, st)
            # sq = [mean_c, var_c + mean_c^2]
            sq = small.tile([128, 2], F32, tag="sq")
            nc.vector.tensor_copy(sq[:, 0:1], mv[:, 0:1])
            nc.vector.tensor_tensor(sq[:, 1:2], mv[:, 0:1], mv[:, 0:1], op=OP.mult)
            nc.vector.tensor_tensor(sq[:, 1:2], sq[:, 1:2], mv[:, 1:2], op=OP.add)
            # group sums over the 8 groups in this chunk
            gs = pmm.tile([8, 2], F32, tag="mm")
            nc.tensor.matmul(gs, lhsT=e_mat, rhs=sq, start=True, stop=True)
            gsb = small.tile([8, 2], F32, tag="gsb")
            nc.vector.tensor_scalar_mul(gsb, gs, 1.0 / GSIZE)  # [m_g, E[x^2]_g]
            var = small.tile([8, 1], F32, tag="var")
            nc.vector.tensor_tensor(var, gsb[:, 0:1], gsb[:, 0:1], op=OP.mult)
            nc.vector.tensor_tensor(var, gsb[:, 1:2], var, op=OP.subtract)
            std = small.tile([8, 1], F32, tag="std")
            nc.scalar.activation(std, var, AF.Sqrt, bias=eps_sb)
            nc.vector.reciprocal(gsb[:, 1:2], std)             # [m_g, rstd_g]
            # broadcast group stats back to channels
            bc = pmm.tile([128, 2], F32, tag="mm")
            nc.tensor.matmul(bc, lhsT=e2_mat, rhs=gsb, start=True, stop=True)
            ab = small.tile([128, 2], F32, tag="ab")
            nc.vector.tensor_tensor(ab[:, 0:1], bc[:, 1:2],
                                    gamma_sb[:, cc:cc + 1], op=OP.mult)
            nc.vector.tensor_tensor(ab[:, 1:2], bc[:, 0:1], ab[:, 0:1], op=OP.mult)
            nc.vector.tensor_tensor(ab[:, 1:2], beta_sb[:, cc:cc + 1],
                                    ab[:, 1:2], op=OP.subtract)
            nc.vector.tensor_scalar(out=xT[:, cc, :], in0=xT[:, cc, :].bitcast(F32),
                                    scalar1=ab[:, 0:1], scalar2=ab[:, 1:2],
                                    op0=OP.mult, op1=OP.add)
        return xT

    def phase_qkv(b, xT):
        """qkT [128, 8, L] bf16 (features f=fi*128+p on partitions; q fi 0-3,
        k fi 4-7) and v [128, tt, 9, 64] bf16 ([tok, head, d]; head slot 8 is
        all-ones so [v_h | ones] forms one strided lhsT per head)."""
        qkT = big.tile([128, 8, L], BF16, tag="qkT")
        v_sb = big.tile([128, NTT, 8, 2 * D], BF16, tag="v")
        for tt in range(NTT):
            nc.vector.memset(v_sb[:, tt, :, D:2 * D], 1.0)
        for fi in range(8):
            for tb in range(2):
                ps = pmm.tile([128, 512], F32, tag="mm")
                for kc in range(NCHUNK):
                    nc.tensor.matmul(
                        ps,
                        lhsT=wq_sb[:, kc, fi * 128:(fi + 1) * 128],
                        rhs=xT[:, kc, tb * 512:(tb + 1) * 512],
                        start=(kc == 0), stop=(kc == NCHUNK - 1),
                    )
                nc.vector.tensor_scalar(
                    out=qkT[:, fi, tb * 512:(tb + 1) * 512], in0=ps,
                    scalar1=bqk_sb[:, fi:fi + 1], scalar2=None, op0=OP.add)
        for tt in range(NTT):
            ps = pmm.tile([128, 512], F32, tag="mm")
            for kc in range(NCHUNK):
                nc.tensor.matmul(
                    ps,
                    lhsT=xT[:, kc, tt * 128:(tt + 1) * 128],
                    rhs=wq_sb[:, kc, 2 * C:3 * C],
                    start=(kc == 0), stop=(kc == NCHUNK - 1),
                )
            nc.vector.tensor_tensor(
                out=v_sb[:, tt, :, 0:D],
                in0=ps.rearrange("p (h d) -> p h d", d=D),
                in1=bv_bc.rearrange("p (h d) -> p h d", d=D), op=OP.add)
        return qkT, v_sb

    def phase_attention(qkT, v_sb):
        """attn_outT [128, chunk, L] fp32: feature f=kc*128+p, head=2*kc+(p>=64)."""
        aT = big.tile([128, NCHUNK, L], BF16, tag="attnT")
        for hp in range(4):
            h0, h1 = 2 * hp, 2 * hp + 1
            qT0 = qkT[0:64, hp, :]
            kT0 = qkT[0:64, 4 + hp, :]
            qT1 = qkT[64:128, hp, :]
            kT1 = qkT[64:128, 4 + hp, :]
            for qb in range(2):
                qs = slice(qb * 512, (qb + 1) * 512)
                out0 = paout.tile([128, 512], F32, tag="aout")
                out1 = paout.tile([128, 512], F32, tag="aout")
                for g in range(4):
                    s0 = pscore.tile([128, 2, 512], F32, tag="sc")
                    s1 = pscore.tile([128, 2, 512], F32, tag="sc")
                    for j in range(2):
                        kt = 2 * g + j
                        ks = slice(kt * 128, (kt + 1) * 128)
                        nc.tensor.matmul(s0[:, j], lhsT=kT0[:, ks], rhs=qT0[:, qs],
                                         start=True, stop=True)
                        nc.tensor.matmul(s1[:, j], lhsT=kT1[:, ks], rhs=qT1[:, qs],
                                         start=True, stop=True)
                    e0 = epool.tile([128, 2, 512], BF16, tag="e")
                    e1 = epool.tile([128, 2, 512], BF16, tag="e")
                    nc.scalar.activation(e0, s0, AF.Exp, scale=SCALE)
                    nc.scalar.activation(e1, s1, AF.Exp, scale=SCALE)
                    for j in range(2):
                        kt = 2 * g + j
                        for (ops, vh, eh) in ((out0, h0, e0), (out1, h1, e1)):
                            # lhsT [128, 128] = [v_h | ones]: out rows 0-63 =
                            # attn@v, rows 64-127 = softmax-denominator
                            # replicas (the ones columns), all in one matmul.
                            nc.tensor.matmul(
                                ops, lhsT=v_sb[:, kt, vh, :],
                                rhs=eh[:, j], start=(kt == 0), stop=(kt == 7))
                den0 = rpool.tile([64, 512], F32, tag="den")
                nc.vector.tensor_copy(den0, out0[64:128])
                rc0 = rpool.tile([64, 512], F32, tag="rc")
                nc.vector.reciprocal_approx_fast(rc0, den0)
                nc.vector.tensor_tensor(out=aT[0:64, hp, qs], in0=out0[0:64],
                                        in1=rc0, op=OP.mult)
                den1 = rpool.tile([64, 512], F32, tag="den")
                nc.vector.tensor_copy(den1, out1[64:128])
                rc1 = rpool.tile([64, 512], F32, tag="rc")
                nc.vector.reciprocal_approx_fast(rc1, den1)
                nc.vector.tensor_tensor(out=aT[64:128, hp, qs], in0=out1[0:64],
                                        in1=rc1, op=OP.mult)
        return aT

    def phase_proj(b, aT):
        for tt in range(NTT):
            ps = pmm.tile([128, 512], F32, tag="mm")
            for kc in range(NCHUNK):
                nc.tensor.matmul(
                    ps,
                    lhsT=aT[:, kc, tt * 128:(tt + 1) * 128],
                    rhs=wo_sb[:, kc, :],
                    start=(kc == 0), stop=(kc == NCHUNK - 1),
                )
            hh = hpool.tile([128, C], F32, tag="h")
            nc.vector.tensor_tensor(out=hh, in0=ps, in1=bo_bc, op=OP.add)
            # residual x is added by the accumulating DMA below (x was
            # pre-copied into out_d at kernel start).
            nc.gpsimd.dma_start(out_d[b, tt * 128:(tt + 1) * 128, :], hh,
                                accum_op=OP.add)

    # Pre-copy x into the output buffer; the projection DMAs accumulate on top.
    for b in range(B_LOCAL):
        nc.sync.dma_start(out_d[b], x_d[b])
    xTs = [phase_transpose_gn(b) for b in range(B_LOCAL)]
    qkvs = [phase_qkv(b, xTs[b]) for b in range(B_LOCAL)]
    for b in range(B_LOCAL):
        aT = phase_attention(*qkvs[b])
        phase_proj(b, aT)


_NC_CACHE = None


def _get_nc():
    global _NC_CACHE
    if _NC_CACHE is None:
        from contextlib import ExitStack

        nc = bacc.Bacc("TRN2", target_bir_lowering=False, debug=False)
        with tile.TileContext(nc) as tc, ExitStack() as ctx:
            build_attention_block(tc, ctx)
        nc.compile()
        _NC_CACHE = nc
    return _NC_CACHE


def run(inputs, trace=False, tmpdir=None):
    """Run on 8 NeuronCores. Returns (full_output, BassKernelResults)."""
    from concourse import bass_utils

    x = np.ascontiguousarray(np.asarray(inputs["x"], dtype=np.float32))
    B, H, W, Cc = x.shape
    xs = x.reshape(B, H * W, Cc)
    common = {
        "gamma": np.ascontiguousarray(np.asarray(inputs["gamma"], np.float32)),
        "beta": np.ascontiguousarray(np.asarray(inputs["beta"], np.float32)),
        "w_qkv": np.ascontiguousarray(np.asarray(inputs["w_qkv"], np.float32)),
        "b_qkv": np.ascontiguousarray(np.asarray(inputs["b_qkv"], np.float32)),
        "w_out": np.ascontiguousarray(np.asarray(inputs["w_out"], np.float32)),
        "b_out": np.ascontiguousarray(np.asarray(inputs["b_out"], np.float32)),
    }
    n_cores = 8
    per = B // n_cores
    in_maps = [
        {"x": np.ascontiguousarray(xs[c * per:(c + 1) * per]), **common}
        for c in range(n_cores)
    ]
    nc = _get_nc()
    res = bass_utils.run_bass_kernel_spmd(
        nc, in_maps, core_ids=list(range(n_cores)), trace=trace, tmpdir=tmpdir)
    out = np.concatenate([r["out"] for r in res.results], axis=0)
    return out.reshape(B, H, W, Cc), res


def kernel(**inputs):
    out, _ = run(inputs, trace=False)
    return out
